# revision 1
# baseline (speedup 1.0000x reference)
"""Trainium2 Bass kernel for nn_Attention_10909216932430.

Reference computation (per sample n of N=8, C=256, HW=4096):
    Q = Wq @ x + bq ; K = Wk @ x + bk          (V computed but unused)
    att = softmax_j(Q^T K)                      [HW, HW]
    out = att @ x^T  -> out[c, i] = sum_j att[i, j] x[c, j]

Algebraic simplification used here:
    S[i,j] = Q^T K = x^T (Wq^T Wk) x + (Wk^T bq)^T x |_j + (Wq^T bk)^T x |_i + bq.bk
Terms independent of j cancel inside softmax_j, so with
    A = Wq^T Wk,  u = Wk^T bq,  w = u^T x   (w is a per-j bias)
    softmax_j(S) == softmax_j(x^T (A x) + w[j])
bk, Wv, bv drop out entirely.  No max-subtraction is needed: |S| < ~40,
comfortably inside fp32 exp range, and softmax is shift-invariant.

Device program (one sample per NeuronCore, data-parallel over N=8):
    B  = Wk^T Wq                  (= A^T, layout [c' part, c])
    u  = Wk^T bq
    xT = transpose(x)             (PE transpose, [j part, c])
    Y  = A x   (via lhsT=B)       ([c part, j])
    wT[j] = u^T x                 ([j part, 1] per 128-chunk)
    for each i-chunk (512) and j-chunk (128):
        ST_psum[j,i] = Y[:,jc]^T x[:,ic]         (2 MMs, accum over c)
        e = exp(ST_psum + wT[jc])                (ACT, bias = per-partition)
        out_psum[c_blk, i] += xT[jc,c_blk]^T e   (2 MMs, accum over jc)
        eacc += e                                (DVE; denominator partials)
    den[1, i] = ones^T eacc                      (1 MM per i-chunk)
    out[c, i] = out_psum * bcast(1 / den)        (DVE mul)

Matmul operand tensors are float32r (the PE's full-rate fp32 mode:
1 cycle/row vs 4 for exact fp32; storage is still 4-byte fp32).  The
main loop is software-pipelined: exp for the pair two ahead is issued
before each pair's PV matmuls, and the per-i-chunk normalization is
issued 2/4 pairs late, so the PE stream never stalls on ACT/DVE.
"""

import numpy as np

import concourse.bass as bass
import concourse.mybir as mybir
import concourse.tile as tile
from concourse import bacc
from concourse import bass_utils
from concourse.bass import ts
from concourse.masks import make_identity

N, C, HW = 8, 256, 4096
P = 128           # partitions
IC = 512          # i-chunk (PSUM bank width in fp32)
NJ = HW // P      # 32 j-chunks of 128
NI = HW // IC     # 8 i-chunks of 512
F32 = mybir.dt.float32
F32R = mybir.dt.float32r
EXP = mybir.ActivationFunctionType.Exp


def build_kernel(nc, tc, out_d, x_d, wq_d, wk_d, bq_d, pv_dt=F32R, repeats=1):
    from contextlib import ExitStack

    with ExitStack() as ctx:
        const = ctx.enter_context(tc.tile_pool(name="const", bufs=1))

        # Persistent SBUF tensors. Channel dim C=256 is split into 2 chunks of 128.
        x_sb = const.tile([P, 2, HW], F32R)    # x[c, j]: [:, cc, :] = rows cc*128..
        xT_sb = const.tile([P, NJ, C], F32R)   # x^T: [j%128, j//128, c]
        y_sb = const.tile([P, 2, HW], F32R)    # Y = A x, same layout as x
        wq_sb = const.tile([P, 2, C], F32R)    # Wq[o, c]: [:, oc, :]
        wk_sb = const.tile([P, 2, C], F32R)
        bq_sb = const.tile([P, 2], F32)        # bq[o]: [o%128, o//128]
        b_sb = const.tile([P, 2, C], F32R)     # B = Wk^T Wq: [c'%128, c'//128, c]
        u_sb = const.tile([P, 2], F32)         # u = Wk^T bq: [c%128, c//128]
        wT_sb = const.tile([P, NJ], F32)       # w^T: [j%128, j//128]
        ones_row = const.tile([1, P], F32R)
        ident = const.tile([P, P], F32R)

        # constants: memset/affine_select lack f32r ISA support -> build in
        # f32 and round-convert on the DVE.
        ones_f = const.tile([P, 1], F32)
        ones_row_f = const.tile([1, P], F32)
        ident_f = const.tile([P, P], F32)
        nc.vector.memset(ones_f, 1.0)
        nc.vector.memset(ones_row_f, 1.0)
        make_identity(nc, ident_f)
        ones_col = const.tile([P, 1], F32R)
        nc.vector.tensor_copy(out=ones_col, in_=ones_f)
        nc.vector.tensor_copy(out=ones_row, in_=ones_row_f)
        nc.vector.tensor_copy(out=ident, in_=ident_f)
        # touch Exp early so the ACT table set loads during the DMA prologue
        warm = const.tile([1, 1], F32)
        nc.scalar.activation(out=warm, in_=ones_f[0:1, 0:1], func=EXP)

        for cc in range(2):
            nc.sync.dma_start(out=wq_sb[:, cc, :], in_=wq_d[ts(cc, P), :])
            nc.sync.dma_start(out=wk_sb[:, cc, :], in_=wk_d[ts(cc, P), :])
        nc.sync.dma_start(out=bq_sb, in_=bq_d)
        for q in range(8):
            for cc in range(2):
                nc.sync.dma_start(
                    out=x_sb[:, cc, ts(q, HW // 8)],
                    in_=x_d[ts(cc, P), ts(q, HW // 8)],
                )

        # --- setup phase 1: B = Wk^T Wq and u = Wk^T bq -------------------
        with tc.tile_pool(name="ps_a", bufs=2, space="PSUM") as ps_a:
            for blk in range(2):
                psb = ps_a.tile([P, C], F32, tag="psb")
                for oc in range(2):
                    nc.tensor.matmul(
                        psb,
                        lhsT=wk_sb[:, oc, ts(blk, P)],
                        rhs=wq_sb[:, oc, :],
                        start=(oc == 0),
                        stop=(oc == 1),
                    )
                nc.scalar.copy(out=b_sb[:, blk, :], in_=psb)
            for blk in range(2):
                psu = ps_a.tile([P, 1], F32, tag="psu")
                for oc in range(2):
                    nc.tensor.matmul(
                        psu,
                        lhsT=wk_sb[:, oc, ts(blk, P)].bitcast(F32),
                        rhs=bq_sb[:, oc : oc + 1],
                        start=(oc == 0),
                        stop=(oc == 1),
                    )
                nc.scalar.copy(out=u_sb[:, blk : blk + 1], in_=psu)

        # --- setup phase 2: xT, Y, wT ------------------------------------
        with tc.tile_pool(name="ps_b", bufs=2, space="PSUM") as ps_b:
            for jc in range(NJ):
                pst = ps_b.tile([P, C], F32R, tag="pst")
                for cc in range(2):
                    nc.tensor.transpose(
                        pst[:, ts(cc, P)], x_sb[:, cc, ts(jc, P)], ident
                    )
                nc.vector.tensor_copy(out=xT_sb[:, jc, :], in_=pst)
            for blk in range(2):
                for jj in range(NI):
                    psy = ps_b.tile([P, IC], F32, tag="psy")
                    for cc in range(2):
                        nc.tensor.matmul(
                            psy,
                            lhsT=b_sb[:, cc, ts(blk, P)],
                            rhs=x_sb[:, cc, ts(jj, IC)],
                            start=(cc == 0),
                            stop=(cc == 1),
                        )
                    nc.scalar.copy(out=y_sb[:, blk, ts(jj, IC)], in_=psy)
            psw = ps_b.tile([P, NJ], F32, tag="psw")
            for jc in range(NJ):
                for cc in range(2):
                    nc.tensor.matmul(
                        psw[:, jc : jc + 1],
                        lhsT=x_sb[:, cc, ts(jc, P)].bitcast(F32),
                        rhs=u_sb[:, cc : cc + 1],
                        start=(cc == 0),
                        stop=(cc == 1),
                    )
            nc.scalar.copy(out=wT_sb, in_=psw)

        xT_bf = None
        if pv_dt != F32R:
            xT_bf = const.tile([P, NJ, C], pv_dt)
            for jc in range(NJ):
                nc.vector.tensor_copy(out=xT_bf[:, jc, :], in_=xT_sb[:, jc, :])

        # --- main loop ----------------------------------------------------
        mains = ctx.enter_context(tc.tile_pool(name="mains", bufs=3))
        outp = ctx.enter_context(tc.tile_pool(name="outp", bufs=3))
        ps_s = ctx.enter_context(tc.tile_pool(name="ps_s", bufs=3, space="PSUM"))
        ps_o = ctx.enter_context(tc.tile_pool(name="ps_o", bufs=4, space="PSUM"))
        ps_e = ctx.enter_context(tc.tile_pool(name="ps_e", bufs=1, space="PSUM"))

        chunks = [(ii * IC, IC) for ii in range(NI)]

        def score_exp(ci, jc):
            """S^T[j128, i_chunk] for (chunk ci, jc), exp'd into SBUF."""
            i0, iw = chunks[ci]
            ps = ps_s.tile([P, IC], F32, tag="ps")
            nc.tensor.matmul(
                ps[:, :iw],
                lhsT=y_sb[:, 0, ts(jc, P)],
                rhs=x_sb[:, 0, i0 : i0 + iw],
                start=True,
                stop=False,
            )
            nc.tensor.matmul(
                ps[:, :iw],
                lhsT=y_sb[:, 1, ts(jc, P)],
                rhs=x_sb[:, 1, i0 : i0 + iw],
                start=False,
                stop=True,
            )
            e = mains.tile([P, IC], pv_dt, tag="e", bufs=4)
            nc.scalar.activation(
                out=e[:, :iw], in_=ps[:, :iw], func=EXP,
                bias=wT_sb[:, jc : jc + 1], scale=1.0,
            )
            return e

        # Flat software pipeline over all (chunk, jc) pairs.  score_exp for
        # the pair TWO ahead is issued before this pair's PV matmuls so the
        # PE never head-of-line blocks on ACT's exp; the per-chunk
        # normalization is split into two stages issued 2 and 4 pairs late
        # so the PE's pden/broadcast matmuls never wait on the DVE chain.
        pairs = [(ci, jc) for ci in range(len(chunks)) for jc in range(NJ)]
        state = {}   # per-chunk: po0, po1, eacc, rden
        due = {}     # idx -> list of stage callables

        def stage_a(ci, _rep=0):
            st = state[ci]
            _, iw = chunks[ci]
            pden = ps_e.tile([1, IC], F32, tag="eps", name=f"pden_{_rep}_{ci}")
            nc.tensor.matmul(
                pden[:, :iw], lhsT=ones_col, rhs=st["eacc"][:, :iw],
                start=True, stop=True,
            )
            rden = mains.tile([1, IC], F32R, tag="rden")
            with nc.allow_low_precision(reason="f32r is reduced-precision fp32"):
                nc.vector.reciprocal(rden[:, :iw], pden[:, :iw])
            st["rden"] = rden

        def stage_b(ci, _rep=0):
            st = state[ci]
            i0, iw = chunks[ci]
            # broadcast recip to all partitions: ones[1,128].T @ rden[1,iw]
            pbc = ps_e.tile([P, IC], F32, tag="eps", name=f"pbc_{_rep}_{ci}")
            nc.tensor.matmul(
                pbc[:, :iw], lhsT=ones_row, rhs=st["rden"][:, :iw],
                start=True, stop=True,
            )
            bc = mains.tile([P, IC], F32, tag="bc")
            nc.vector.tensor_copy(out=bc[:, :iw], in_=pbc[:, :iw])
            o0 = outp.tile([P, IC], F32, tag="o")
            o1 = outp.tile([P, IC], F32, tag="o")
            nc.vector.tensor_mul(o0[:, :iw], st["po0"][:, :iw], bc[:, :iw])
            nc.vector.tensor_mul(o1[:, :iw], st["po1"][:, :iw], bc[:, :iw])
            nc.sync.dma_start(out=out_d[0:P, i0 : i0 + iw], in_=o0[:, :iw])
            nc.sync.dma_start(out=out_d[P:C, i0 : i0 + iw], in_=o1[:, :iw])
            del state[ci]

        for _rep in range(repeats):
          e_queue = [score_exp(*pairs[0]), score_exp(*pairs[1])]
          for idx, (ci, jc) in enumerate(pairs):
              iw = chunks[ci][1]
              if jc == 0:
                  state[ci] = {
                      "po0": ps_o.tile([P, IC], F32, tag="po", name=f"po0_{_rep}_{ci}"),
                      "po1": ps_o.tile([P, IC], F32, tag="po", name=f"po1_{_rep}_{ci}"),
                      "eacc": mains.tile(
                          [P, IC], F32R, tag="eacc", bufs=2, name=f"eacc_{_rep}_{ci}"
                      ),
                  }
              st = state[ci]
              if idx + 2 < len(pairs):
                  e_queue.append(score_exp(*pairs[idx + 2]))
              e_cur = e_queue.pop(0)
              first, last = jc == 0, jc == NJ - 1
              xT_l = xT_sb if pv_dt == F32R else xT_bf
              nc.tensor.matmul(
                  st["po0"][:, :iw], lhsT=xT_l[:, jc, 0:P], rhs=e_cur[:, :iw],
                  start=first, stop=last,
              )
              nc.tensor.matmul(
                  st["po1"][:, :iw], lhsT=xT_l[:, jc, P:C], rhs=e_cur[:, :iw],
                  start=first, stop=last,
              )
              # denominator partials accumulate on the DVE (keeps the PE at
              # 4 matmuls per tile-pair); stage_a's ones-matmul folds the
              # partitions once per chunk.
              e_rd = e_cur[:, :iw].bitcast(F32) if pv_dt == F32R else e_cur[:, :iw]
              if first:
                  nc.vector.tensor_copy(out=st["eacc"][:, :iw], in_=e_rd)
              else:
                  nc.vector.tensor_add(st["eacc"][:, :iw], st["eacc"][:, :iw], e_rd)
              if last:
                  due.setdefault(idx + 2, []).append(lambda ci=ci, r=_rep: stage_a(ci, r))
                  due.setdefault(idx + 4, []).append(lambda ci=ci, r=_rep: stage_b(ci, r))
              for fn in due.pop(idx, []):
                  fn()
          for idx in sorted(due):
              for fn in due[idx]:
                  fn()
          due.clear()


_NC_CACHE = {}


def _get_nc(pv_dt=F32R, repeats=1):
    key = (pv_dt, repeats)
    if key in _NC_CACHE:
        return _NC_CACHE[key]
    nc = bacc.Bacc(
        "TRN2",
        target_bir_lowering=False,
        debug=False,
        enable_asserts=False,
        num_devices=N,
    )
    x_d = nc.dram_tensor("x", [C, HW], F32R, kind="ExternalInput").ap()
    wq_d = nc.dram_tensor("wq", [C, C], F32R, kind="ExternalInput").ap()
    wk_d = nc.dram_tensor("wk", [C, C], F32R, kind="ExternalInput").ap()
    bq_d = nc.dram_tensor("bq", [P, 2], F32, kind="ExternalInput").ap()
    out_d = nc.dram_tensor("out", [C, HW], F32, kind="ExternalOutput").ap()
    with tile.TileContext(nc) as tc:
        build_kernel(nc, tc, out_d, x_d, wq_d, wk_d, bq_d, pv_dt=pv_dt,
                     repeats=repeats)
    nc.compile()
    _NC_CACHE[key] = nc
    return nc


def make_in_maps(batch_flat, Wq, bq, Wk):
    bq_r = np.ascontiguousarray(
        np.asarray(bq, dtype=np.float32).reshape(2, P).T
    )
    wq = np.ascontiguousarray(np.asarray(Wq, dtype=np.float32))
    wk = np.ascontiguousarray(np.asarray(Wk, dtype=np.float32))
    return [
        {
            "x": np.ascontiguousarray(np.asarray(batch_flat[n], dtype=np.float32)),
            "wq": wq,
            "wk": wk,
            "bq": bq_r,
        }
        for n in range(N)
    ]


def kernel(batch_flat, Wq, bq, Wk, bk=None, Wv=None, bv=None, **_unused):
    nc = _get_nc()
    in_maps = make_in_maps(batch_flat, Wq, bq, Wk)
    res = bass_utils.run_bass_kernel_spmd(nc, in_maps, core_ids=list(range(N)))
    return np.stack([res.results[n]["out"] for n in range(N)])



# revision 10
# speedup vs baseline: 2.8280x; 2.8280x over previous
"""Trainium2 Bass kernel for nn_Attention_10909216932430.

Reference computation (per sample n of N=8, C=256, HW=4096):
    Q = Wq @ x + bq ; K = Wk @ x + bk          (V computed but unused)
    att = softmax_j(Q^T K)                      [HW, HW]
    out = att @ x^T  -> out[c, i] = sum_j att[i, j] x[c, j]

Algebraic simplification used here:
    S[i,j] = Q^T K = x^T (Wq^T Wk) x + (Wk^T bq)^T x |_j + (Wq^T bk)^T x |_i + bq.bk
Terms independent of j cancel inside softmax_j, so with
    A = Wq^T Wk,  u = Wk^T bq,  w = u^T x   (w is a per-j bias)
    softmax_j(S) == softmax_j(x^T (A x) + w[j])
bk, Wv, bv drop out entirely.  No max-subtraction is needed: |S| < ~40,
comfortably inside fp32 exp range, and softmax is shift-invariant.

Device program (one sample per NeuronCore, data-parallel over N=8):
    B  = Wk^T Wq                  (= A^T, layout [c' part, c])
    u  = Wk^T bq
    xT = transpose(x)             (PE transpose, [j part, c])
    Y  = A x   (via lhsT=B)       ([c part, j])
    wT[j] = u^T x                 ([j part, 1] per 128-chunk)
    for each i-chunk (512) and j-chunk (128):
        ST_psum[j,i] = Y[:,jc]^T x[:,ic]         (2 MMs, accum over c)
        e = exp(ST_psum + wT[jc])                (ACT, bias = per-partition)
        out_psum[c_blk, i] += xT[jc,c_blk]^T e   (2 MMs, accum over jc)
        eacc += e                                (DVE; denominator partials)
    allden[p, i] = sum_p eacc                    (GPSIMD partition_all_reduce,
                                                  fold + bcast in one op)
    out[c, i] = out_psum * (1 / allden)          (DVE recip + mul)

Matmul operand tensors are float32r (the PE's full-rate fp32 mode:
1 cycle/row vs 4 for exact fp32; storage is still 4-byte fp32).  The
main loop is software-pipelined: exp for the pair two ahead is issued
before each pair's PV matmuls, and the per-i-chunk normalization is
issued 2/6 pairs late, so the PE stream never stalls on ACT/DVE.  The
denominator fold/broadcast runs on the otherwise-idle GPSIMD engine so
the PE does nothing per rep except the 1024 main matmuls.
"""

import numpy as np

import concourse.bass as bass
import concourse.bass_isa as bass_isa
import concourse.mybir as mybir
import concourse.tile as tile
from concourse import bacc
from concourse import bass_utils
from concourse.bass import ts
from concourse.masks import make_identity

N, C, HW = 8, 256, 4096
P = 128           # partitions
IC = 512          # i-chunk (PSUM bank width in fp32)
NJ = HW // P      # 32 j-chunks of 128
NI = HW // IC     # 8 i-chunks of 512
F32 = mybir.dt.float32
F32R = mybir.dt.float32r
EXP = mybir.ActivationFunctionType.Exp


def build_kernel(nc, tc, out_d, x_d, wq_d, wk_d, bq_d, pv_dt=F32R, repeats=1):
    from contextlib import ExitStack

    with ExitStack() as ctx:
        const = ctx.enter_context(tc.tile_pool(name="const", bufs=1))

        # Persistent SBUF tensors. Channel dim C=256 is split into 2 chunks of 128.
        x_sb = const.tile([P, 2, HW], F32R)    # x[c, j]: [:, cc, :] = rows cc*128..
        xT_sb = const.tile([P, NJ, C], F32R)   # x^T: [j%128, j//128, c]
        y_sb = const.tile([P, 2, HW], F32R)    # Y = A x, same layout as x
        wq_sb = const.tile([P, 2, C], F32R)    # Wq[o, c]: [:, oc, :]
        wk_sb = const.tile([P, 2, C], F32R)
        bq_sb = const.tile([P, 2], F32)        # bq[o]: [o%128, o//128]
        b_sb = const.tile([P, 2, C], F32R)     # B = Wk^T Wq: [c'%128, c'//128, c]
        u_sb = const.tile([P, 2], F32)         # u = Wk^T bq: [c%128, c//128]
        wT_sb = const.tile([P, NJ], F32)       # w^T: [j%128, j//128]
        ident = const.tile([P, P], F32R)

        # constants: memset/affine_select lack f32r ISA support -> build in
        # f32 and round-convert on the DVE.
        ones_f = const.tile([P, 1], F32)
        ident_f = const.tile([P, P], F32)
        nc.vector.memset(ones_f, 1.0)
        make_identity(nc, ident_f)
        nc.vector.tensor_copy(out=ident, in_=ident_f)
        # touch Exp early so the ACT table set loads during the DMA prologue
        warm = const.tile([1, 1], F32)
        nc.scalar.activation(out=warm, in_=ones_f[0:1, 0:1], func=EXP)

        for cc in range(2):
            nc.sync.dma_start(out=wq_sb[:, cc, :], in_=wq_d[ts(cc, P), :])
            nc.sync.dma_start(out=wk_sb[:, cc, :], in_=wk_d[ts(cc, P), :])
        nc.sync.dma_start(out=bq_sb, in_=bq_d)
        for q in range(8):
            for cc in range(2):
                nc.sync.dma_start(
                    out=x_sb[:, cc, ts(q, HW // 8)],
                    in_=x_d[ts(cc, P), ts(q, HW // 8)],
                )

        # --- setup phase 1: B = Wk^T Wq and u = Wk^T bq -------------------
        with tc.tile_pool(name="ps_a", bufs=2, space="PSUM") as ps_a:
            for blk in range(2):
                psb = ps_a.tile([P, C], F32, tag="psb")
                for oc in range(2):
                    nc.tensor.matmul(
                        psb,
                        lhsT=wk_sb[:, oc, ts(blk, P)],
                        rhs=wq_sb[:, oc, :],
                        start=(oc == 0),
                        stop=(oc == 1),
                    )
                nc.scalar.copy(out=b_sb[:, blk, :], in_=psb)
            for blk in range(2):
                psu = ps_a.tile([P, 1], F32, tag="psu")
                for oc in range(2):
                    nc.tensor.matmul(
                        psu,
                        lhsT=wk_sb[:, oc, ts(blk, P)].bitcast(F32),
                        rhs=bq_sb[:, oc : oc + 1],
                        start=(oc == 0),
                        stop=(oc == 1),
                    )
                nc.scalar.copy(out=u_sb[:, blk : blk + 1], in_=psu)

        # --- setup phase 2: xT, Y, wT ------------------------------------
        with tc.tile_pool(name="ps_b", bufs=2, space="PSUM") as ps_b:
            for jc in range(NJ):
                pst = ps_b.tile([P, C], F32R, tag="pst")
                for cc in range(2):
                    nc.tensor.transpose(
                        pst[:, ts(cc, P)], x_sb[:, cc, ts(jc, P)], ident
                    )
                nc.vector.tensor_copy(out=xT_sb[:, jc, :], in_=pst)
            for blk in range(2):
                for jj in range(NI):
                    psy = ps_b.tile([P, IC], F32, tag="psy")
                    for cc in range(2):
                        nc.tensor.matmul(
                            psy,
                            lhsT=b_sb[:, cc, ts(blk, P)],
                            rhs=x_sb[:, cc, ts(jj, IC)],
                            start=(cc == 0),
                            stop=(cc == 1),
                        )
                    nc.scalar.copy(out=y_sb[:, blk, ts(jj, IC)], in_=psy)
            psw = ps_b.tile([P, NJ], F32, tag="psw")
            for jc in range(NJ):
                for cc in range(2):
                    nc.tensor.matmul(
                        psw[:, jc : jc + 1],
                        lhsT=x_sb[:, cc, ts(jc, P)].bitcast(F32),
                        rhs=u_sb[:, cc : cc + 1],
                        start=(cc == 0),
                        stop=(cc == 1),
                    )
            nc.scalar.copy(out=wT_sb, in_=psw)

        xT_bf = None
        if pv_dt != F32R:
            xT_bf = const.tile([P, NJ, C], pv_dt)
            for jc in range(NJ):
                nc.vector.tensor_copy(out=xT_bf[:, jc, :], in_=xT_sb[:, jc, :])

        # --- main loop ----------------------------------------------------
        mains = ctx.enter_context(tc.tile_pool(name="mains", bufs=3))
        outp = ctx.enter_context(tc.tile_pool(name="outp", bufs=3))
        ps_s = ctx.enter_context(tc.tile_pool(name="ps_s", bufs=4, space="PSUM"))
        ps_o = ctx.enter_context(tc.tile_pool(name="ps_o", bufs=4, space="PSUM"))

        chunks = [(ii * IC, IC) for ii in range(NI)]

        def score_exp(ci, jc):
            """S^T[j128, i_chunk] for (chunk ci, jc), exp'd into SBUF."""
            i0, iw = chunks[ci]
            ps = ps_s.tile([P, IC], F32, tag="ps")
            nc.tensor.matmul(
                ps[:, :iw],
                lhsT=y_sb[:, 0, ts(jc, P)],
                rhs=x_sb[:, 0, i0 : i0 + iw],
                start=True,
                stop=False,
            )
            nc.tensor.matmul(
                ps[:, :iw],
                lhsT=y_sb[:, 1, ts(jc, P)],
                rhs=x_sb[:, 1, i0 : i0 + iw],
                start=False,
                stop=True,
            )
            e = mains.tile([P, IC], pv_dt, tag="e", bufs=4)
            nc.scalar.activation(
                out=e[:, :iw], in_=ps[:, :iw], func=EXP,
                bias=wT_sb[:, jc : jc + 1], scale=1.0,
            )
            return e

        # Flat software pipeline over all (chunk, jc) pairs.  score_exp for
        # the pair TWO ahead is issued before this pair's PV matmuls so the
        # PE never head-of-line blocks on ACT's exp; the per-chunk
        # normalization is split into two stages issued 2 and 6 pairs late
        # (GPSIMD fold, then DVE recip+mul once the fold is surely done).
        pairs = [(ci, jc) for ci in range(len(chunks)) for jc in range(NJ)]
        state = {}   # per-chunk: po0, po1, eacc, allden
        due = {}     # idx -> list of stage callables

        def stage_a(ci, _rep=0):
            # Fold the denominator partials over partitions AND broadcast the
            # sum back to all 128 partitions in one GPSIMD op — no PE work.
            st = state[ci]
            _, iw = chunks[ci]
            allden = mains.tile([P, IC], F32, tag="allden")
            nc.gpsimd.partition_all_reduce(
                allden[:, :iw], st["eacc"][:, :iw], channels=P,
                reduce_op=bass_isa.ReduceOp.add,
            )
            st["allden"] = allden

        def stage_b(ci, _rep=0):
            st = state[ci]
            i0, iw = chunks[ci]
            rbc = mains.tile([P, IC], F32R, tag="rbc")
            with nc.allow_low_precision(reason="f32r is reduced-precision fp32"):
                nc.vector.reciprocal(rbc[:, :iw], st["allden"][:, :iw])
            o0 = outp.tile([P, IC], F32, tag="o")
            o1 = outp.tile([P, IC], F32, tag="o")
            nc.vector.tensor_mul(o0[:, :iw], st["po0"][:, :iw], rbc[:, :iw])
            nc.vector.tensor_mul(o1[:, :iw], st["po1"][:, :iw], rbc[:, :iw])
            nc.sync.dma_start(out=out_d[0:P, i0 : i0 + iw], in_=o0[:, :iw])
            nc.sync.dma_start(out=out_d[P:C, i0 : i0 + iw], in_=o1[:, :iw])
            del state[ci]

        for _rep in range(repeats):
          e_queue = [score_exp(*pairs[0]), score_exp(*pairs[1])]
          for idx, (ci, jc) in enumerate(pairs):
              iw = chunks[ci][1]
              if jc == 0:
                  state[ci] = {
                      "po0": ps_o.tile([P, IC], F32, tag="po", name=f"po0_{_rep}_{ci}"),
                      "po1": ps_o.tile([P, IC], F32, tag="po", name=f"po1_{_rep}_{ci}"),
                      "eacc": mains.tile(
                          [P, IC], F32, tag="eacc", bufs=2, name=f"eacc_{_rep}_{ci}"
                      ),
                  }
              st = state[ci]
              if idx + 2 < len(pairs):
                  e_queue.append(score_exp(*pairs[idx + 2]))
              e_cur = e_queue.pop(0)
              first, last = jc == 0, jc == NJ - 1
              xT_l = xT_sb if pv_dt == F32R else xT_bf
              nc.tensor.matmul(
                  st["po0"][:, :iw], lhsT=xT_l[:, jc, 0:P], rhs=e_cur[:, :iw],
                  start=first, stop=last,
              )
              nc.tensor.matmul(
                  st["po1"][:, :iw], lhsT=xT_l[:, jc, P:C], rhs=e_cur[:, :iw],
                  start=first, stop=last,
              )
              # denominator partials accumulate on the DVE (keeps the PE at
              # 4 matmuls per tile-pair); stage_a's GPSIMD all-reduce folds
              # the partitions once per chunk.
              e_rd = e_cur[:, :iw].bitcast(F32) if pv_dt == F32R else e_cur[:, :iw]
              if first:
                  nc.vector.tensor_copy(out=st["eacc"][:, :iw], in_=e_rd)
              else:
                  nc.vector.tensor_add(st["eacc"][:, :iw], st["eacc"][:, :iw], e_rd)
              if last:
                  due.setdefault(idx + 2, []).append(lambda ci=ci, r=_rep: stage_a(ci, r))
                  # stage_b 6 pairs late: the DVE is in-order, so its recip
                  # must not enqueue until the GPSIMD all-reduce (~2.4us) is
                  # surely done, or it head-of-line-blocks the eacc adds.
                  due.setdefault(idx + 6, []).append(lambda ci=ci, r=_rep: stage_b(ci, r))
              for fn in due.pop(idx, []):
                  fn()
          for idx in sorted(due):
              for fn in due[idx]:
                  fn()
          due.clear()


_NC_CACHE = {}


def _get_nc(pv_dt=F32R, repeats=1):
    key = (pv_dt, repeats)
    if key in _NC_CACHE:
        return _NC_CACHE[key]
    nc = bacc.Bacc(
        "TRN2",
        target_bir_lowering=False,
        debug=False,
        enable_asserts=False,
        num_devices=N,
    )
    x_d = nc.dram_tensor("x", [C, HW], F32R, kind="ExternalInput").ap()
    wq_d = nc.dram_tensor("wq", [C, C], F32R, kind="ExternalInput").ap()
    wk_d = nc.dram_tensor("wk", [C, C], F32R, kind="ExternalInput").ap()
    bq_d = nc.dram_tensor("bq", [P, 2], F32, kind="ExternalInput").ap()
    out_d = nc.dram_tensor("out", [C, HW], F32, kind="ExternalOutput").ap()
    with tile.TileContext(nc) as tc:
        build_kernel(nc, tc, out_d, x_d, wq_d, wk_d, bq_d, pv_dt=pv_dt,
                     repeats=repeats)
    nc.compile()
    _NC_CACHE[key] = nc
    return nc


def make_in_maps(batch_flat, Wq, bq, Wk):
    bq_r = np.ascontiguousarray(
        np.asarray(bq, dtype=np.float32).reshape(2, P).T
    )
    wq = np.ascontiguousarray(np.asarray(Wq, dtype=np.float32))
    wk = np.ascontiguousarray(np.asarray(Wk, dtype=np.float32))
    return [
        {
            "x": np.ascontiguousarray(np.asarray(batch_flat[n], dtype=np.float32)),
            "wq": wq,
            "wk": wk,
            "bq": bq_r,
        }
        for n in range(N)
    ]


def kernel(batch_flat, Wq, bq, Wk, bk=None, Wv=None, bv=None, **_unused):
    nc = _get_nc()
    in_maps = make_in_maps(batch_flat, Wq, bq, Wk)
    res = bass_utils.run_bass_kernel_spmd(nc, in_maps, core_ids=list(range(N)))
    return np.stack([res.results[n]["out"] for n in range(N)])



# revision 14
# speedup vs baseline: 3.0717x; 1.0862x over previous
"""Trainium2 Bass kernel for nn_Attention_10909216932430.

Reference computation (per sample n of N=8, C=256, HW=4096):
    Q = Wq @ x + bq ; K = Wk @ x + bk          (V computed but unused)
    att = softmax_j(Q^T K)                      [HW, HW]
    out = att @ x^T  -> out[c, i] = sum_j att[i, j] x[c, j]

Algebraic simplification used here:
    S[i,j] = Q^T K = x^T (Wq^T Wk) x + (Wk^T bq)^T x |_j + (Wq^T bk)^T x |_i + bq.bk
Terms independent of j cancel inside softmax_j, so with
    A = Wq^T Wk,  u = Wk^T bq,  w = u^T x   (w is a per-j bias)
    softmax_j(S) == softmax_j(x^T (A x) + w[j])
bk, Wv, bv drop out entirely.  No max-subtraction is needed: |S| < ~40,
comfortably inside fp32 exp range, and softmax is shift-invariant.

Device program (one sample per NeuronCore, data-parallel over N=8):
    B  = Wk^T Wq                  (= A^T, layout [c' part, c])
    u  = Wk^T bq
    xT = transpose(x)             (PE transpose, [j part, c])
    Y  = A x   (via lhsT=B)       ([c part, j])
    wT[j] = u^T x                 ([j part, 1] per 128-chunk)
    for each i-chunk (512) and j-chunk (128):
        ST_psum[j,i] = Y[:,jc]^T x[:,ic]         (2 MMs, accum over c)
        e = exp(ST_psum + wT[jc])                (ACT, bias = per-partition)
        out_psum[c_blk, i] += xT[jc,c_blk]^T e   (2 MMs, accum over jc)
        eacc += e                                (DVE; denominator partials)
    allden[p, i] = sum_p eacc                    (GPSIMD partition_all_reduce,
                                                  fold + bcast in one op)
    out[c, i] = out_psum * (1 / allden)          (DVE recip + mul)

Matmul operand tensors are float32r (the PE's full-rate fp32 mode:
1 cycle/row vs 4 for exact fp32; storage is still 4-byte fp32).  The
main loop is software-pipelined: exp for the pair three ahead is issued
before each pair's PV matmuls, and the per-i-chunk normalization is
issued 2/6 pairs late, so the PE stream never stalls on ACT/DVE.  The
denominator fold/broadcast runs on the otherwise-idle GPSIMD engine so
the PE does nothing per rep except the 1024 main matmuls.
"""

import numpy as np

import concourse.bass as bass
import concourse.bass_isa as bass_isa
import concourse.mybir as mybir
import concourse.tile as tile
from concourse import bacc
from concourse import bass_utils
from concourse.bass import ts
from concourse.masks import make_identity

N, C, HW = 8, 256, 4096
P = 128           # partitions
IC = 512          # i-chunk (PSUM bank width in fp32)
NJ = HW // P      # 32 j-chunks of 128
NI = HW // IC     # 8 i-chunks of 512
F32 = mybir.dt.float32
F32R = mybir.dt.float32r
EXP = mybir.ActivationFunctionType.Exp


def build_kernel(nc, tc, out_d, x_d, wq_d, wk_d, bq_d, pv_dt=F32R, repeats=1):
    from contextlib import ExitStack

    with ExitStack() as ctx:
        const = ctx.enter_context(tc.tile_pool(name="const", bufs=1))

        # Persistent SBUF tensors. Channel dim C=256 is split into 2 chunks of 128.
        x_sb = const.tile([P, 2, HW], F32R)    # x[c, j]: [:, cc, :] = rows cc*128..
        xT_sb = const.tile([P, NJ, C], F32R)   # x^T: [j%128, j//128, c]
        y_sb = const.tile([P, 2, HW], F32R)    # Y = A x, same layout as x
        wq_sb = const.tile([P, 2, C], F32R)    # Wq[o, c]: [:, oc, :]
        wk_sb = const.tile([P, 2, C], F32R)
        bq_sb = const.tile([P, 2], F32)        # bq[o]: [o%128, o//128]
        b_sb = const.tile([P, 2, C], F32R)     # B = Wk^T Wq: [c'%128, c'//128, c]
        u_sb = const.tile([P, 2], F32)         # u = Wk^T bq: [c%128, c//128]
        wT_sb = const.tile([P, NJ], F32)       # w^T: [j%128, j//128]
        ident = const.tile([P, P], F32R)

        # constants: memset/affine_select lack f32r ISA support -> build in
        # f32 and round-convert on the DVE.
        ones_f = const.tile([P, 1], F32)
        ident_f = const.tile([P, P], F32)
        nc.vector.memset(ones_f, 1.0)
        make_identity(nc, ident_f)
        nc.vector.tensor_copy(out=ident, in_=ident_f)
        # touch Exp early so the ACT table set loads during the DMA prologue
        warm = const.tile([1, 1], F32)
        nc.scalar.activation(out=warm, in_=ones_f[0:1, 0:1], func=EXP)

        for cc in range(2):
            nc.sync.dma_start(out=wq_sb[:, cc, :], in_=wq_d[ts(cc, P), :])
            nc.sync.dma_start(out=wk_sb[:, cc, :], in_=wk_d[ts(cc, P), :])
        nc.sync.dma_start(out=bq_sb, in_=bq_d)
        for q in range(8):
            for cc in range(2):
                nc.sync.dma_start(
                    out=x_sb[:, cc, ts(q, HW // 8)],
                    in_=x_d[ts(cc, P), ts(q, HW // 8)],
                )

        # --- setup phase 1: B = Wk^T Wq and u = Wk^T bq -------------------
        with tc.tile_pool(name="ps_a", bufs=2, space="PSUM") as ps_a:
            for blk in range(2):
                psb = ps_a.tile([P, C], F32, tag="psb")
                for oc in range(2):
                    nc.tensor.matmul(
                        psb,
                        lhsT=wk_sb[:, oc, ts(blk, P)],
                        rhs=wq_sb[:, oc, :],
                        start=(oc == 0),
                        stop=(oc == 1),
                    )
                nc.scalar.copy(out=b_sb[:, blk, :], in_=psb)
            for blk in range(2):
                psu = ps_a.tile([P, 1], F32, tag="psu")
                for oc in range(2):
                    nc.tensor.matmul(
                        psu,
                        lhsT=wk_sb[:, oc, ts(blk, P)].bitcast(F32),
                        rhs=bq_sb[:, oc : oc + 1],
                        start=(oc == 0),
                        stop=(oc == 1),
                    )
                nc.scalar.copy(out=u_sb[:, blk : blk + 1], in_=psu)

        # --- setup phase 2: xT, Y, wT ------------------------------------
        with tc.tile_pool(name="ps_b", bufs=2, space="PSUM") as ps_b:
            for jc in range(NJ):
                pst = ps_b.tile([P, C], F32R, tag="pst")
                for cc in range(2):
                    nc.tensor.transpose(
                        pst[:, ts(cc, P)], x_sb[:, cc, ts(jc, P)], ident
                    )
                nc.vector.tensor_copy(out=xT_sb[:, jc, :], in_=pst)
            for blk in range(2):
                for jj in range(NI):
                    psy = ps_b.tile([P, IC], F32, tag="psy")
                    for cc in range(2):
                        nc.tensor.matmul(
                            psy,
                            lhsT=b_sb[:, cc, ts(blk, P)],
                            rhs=x_sb[:, cc, ts(jj, IC)],
                            start=(cc == 0),
                            stop=(cc == 1),
                        )
                    nc.scalar.copy(out=y_sb[:, blk, ts(jj, IC)], in_=psy)
            psw = ps_b.tile([P, NJ], F32, tag="psw")
            for jc in range(NJ):
                for cc in range(2):
                    nc.tensor.matmul(
                        psw[:, jc : jc + 1],
                        lhsT=x_sb[:, cc, ts(jc, P)].bitcast(F32),
                        rhs=u_sb[:, cc : cc + 1],
                        start=(cc == 0),
                        stop=(cc == 1),
                    )
            nc.scalar.copy(out=wT_sb, in_=psw)

        xT_bf = None
        if pv_dt != F32R:
            xT_bf = const.tile([P, NJ, C], pv_dt)
            for jc in range(NJ):
                nc.vector.tensor_copy(out=xT_bf[:, jc, :], in_=xT_sb[:, jc, :])

        # --- main loop ----------------------------------------------------
        mains = ctx.enter_context(tc.tile_pool(name="mains", bufs=3))
        outp = ctx.enter_context(tc.tile_pool(name="outp", bufs=3))
        ps_s = ctx.enter_context(tc.tile_pool(name="ps_s", bufs=4, space="PSUM"))
        ps_o = ctx.enter_context(tc.tile_pool(name="ps_o", bufs=4, space="PSUM"))

        chunks = [(ii * IC, IC) for ii in range(NI)]

        def score_exp(ci, jc):
            """S^T[j128, i_chunk] for (chunk ci, jc), exp'd into SBUF."""
            i0, iw = chunks[ci]
            ps = ps_s.tile([P, IC], F32, tag="ps")
            nc.tensor.matmul(
                ps[:, :iw],
                lhsT=y_sb[:, 0, ts(jc, P)],
                rhs=x_sb[:, 0, i0 : i0 + iw],
                start=True,
                stop=False,
            )
            nc.tensor.matmul(
                ps[:, :iw],
                lhsT=y_sb[:, 1, ts(jc, P)],
                rhs=x_sb[:, 1, i0 : i0 + iw],
                start=False,
                stop=True,
            )
            e = mains.tile([P, IC], pv_dt, tag="e", bufs=6)
            nc.scalar.activation(
                out=e[:, :iw], in_=ps[:, :iw], func=EXP,
                bias=wT_sb[:, jc : jc + 1], scale=1.0,
            )
            return e

        # Flat software pipeline over all (chunk, jc) pairs.  score_exp for
        # the pair TWO ahead is issued before this pair's PV matmuls so the
        # PE never head-of-line blocks on ACT's exp; the per-chunk
        # normalization is split into two stages issued 2 and 6 pairs late
        # (GPSIMD fold, then DVE recip+mul once the fold is surely done).
        pairs = [(ci, jc) for ci in range(len(chunks)) for jc in range(NJ)]
        state = {}   # per-chunk: po0, po1, eacc, allden
        due = {}     # idx -> list of stage callables

        def stage_a(ci, _rep=0):
            # Fold the denominator partials over partitions AND broadcast the
            # sum back to all 128 partitions in one GPSIMD op — no PE work.
            st = state[ci]
            _, iw = chunks[ci]
            allden = mains.tile([P, IC], F32, tag="allden")
            nc.gpsimd.partition_all_reduce(
                allden[:, :iw], st["eacc"][:, :iw], channels=P,
                reduce_op=bass_isa.ReduceOp.add,
            )
            st["allden"] = allden

        def stage_b(ci, _rep=0):
            st = state[ci]
            i0, iw = chunks[ci]
            rbc = mains.tile([P, IC], F32R, tag="rbc")
            with nc.allow_low_precision(reason="f32r is reduced-precision fp32"):
                nc.vector.reciprocal(rbc[:, :iw], st["allden"][:, :iw])
            o0 = outp.tile([P, IC], F32, tag="o")
            o1 = outp.tile([P, IC], F32, tag="o")
            nc.vector.tensor_mul(o0[:, :iw], st["po0"][:, :iw], rbc[:, :iw])
            nc.vector.tensor_mul(o1[:, :iw], st["po1"][:, :iw], rbc[:, :iw])
            nc.sync.dma_start(out=out_d[0:P, i0 : i0 + iw], in_=o0[:, :iw])
            nc.sync.dma_start(out=out_d[P:C, i0 : i0 + iw], in_=o1[:, :iw])
            del state[ci]

        for _rep in range(repeats):
          e_queue = [score_exp(*pairs[k]) for k in range(3)]
          for idx, (ci, jc) in enumerate(pairs):
              iw = chunks[ci][1]
              if jc == 0:
                  state[ci] = {
                      "po0": ps_o.tile([P, IC], F32, tag="po", name=f"po0_{_rep}_{ci}"),
                      "po1": ps_o.tile([P, IC], F32, tag="po", name=f"po1_{_rep}_{ci}"),
                      "eacc": mains.tile(
                          [P, IC], F32, tag="eacc", bufs=2, name=f"eacc_{_rep}_{ci}"
                      ),
                  }
              st = state[ci]
              if idx + 3 < len(pairs):
                  e_queue.append(score_exp(*pairs[idx + 3]))
              e_cur = e_queue.pop(0)
              first, last = jc == 0, jc == NJ - 1
              xT_l = xT_sb if pv_dt == F32R else xT_bf
              nc.tensor.matmul(
                  st["po0"][:, :iw], lhsT=xT_l[:, jc, 0:P], rhs=e_cur[:, :iw],
                  start=first, stop=last,
              )
              nc.tensor.matmul(
                  st["po1"][:, :iw], lhsT=xT_l[:, jc, P:C], rhs=e_cur[:, :iw],
                  start=first, stop=last,
              )
              # denominator partials accumulate on the DVE (keeps the PE at
              # 4 matmuls per tile-pair); stage_a's GPSIMD all-reduce folds
              # the partitions once per chunk.
              e_rd = e_cur[:, :iw].bitcast(F32) if pv_dt == F32R else e_cur[:, :iw]
              if first:
                  nc.vector.tensor_copy(out=st["eacc"][:, :iw], in_=e_rd)
              else:
                  nc.vector.tensor_add(st["eacc"][:, :iw], st["eacc"][:, :iw], e_rd)
              if last:
                  due.setdefault(idx + 2, []).append(lambda ci=ci, r=_rep: stage_a(ci, r))
                  # stage_b 6 pairs late: the DVE is in-order, so its recip
                  # must not enqueue until the GPSIMD all-reduce (~2.4us) is
                  # surely done, or it head-of-line-blocks the eacc adds.
                  due.setdefault(idx + 6, []).append(lambda ci=ci, r=_rep: stage_b(ci, r))
              for fn in due.pop(idx, []):
                  fn()
          for idx in sorted(due):
              for fn in due[idx]:
                  fn()
          due.clear()


_NC_CACHE = {}


def _get_nc(pv_dt=F32R, repeats=1):
    key = (pv_dt, repeats)
    if key in _NC_CACHE:
        return _NC_CACHE[key]
    nc = bacc.Bacc(
        "TRN2",
        target_bir_lowering=False,
        debug=False,
        enable_asserts=False,
        num_devices=N,
    )
    x_d = nc.dram_tensor("x", [C, HW], F32R, kind="ExternalInput").ap()
    wq_d = nc.dram_tensor("wq", [C, C], F32R, kind="ExternalInput").ap()
    wk_d = nc.dram_tensor("wk", [C, C], F32R, kind="ExternalInput").ap()
    bq_d = nc.dram_tensor("bq", [P, 2], F32, kind="ExternalInput").ap()
    out_d = nc.dram_tensor("out", [C, HW], F32, kind="ExternalOutput").ap()
    with tile.TileContext(nc) as tc:
        build_kernel(nc, tc, out_d, x_d, wq_d, wk_d, bq_d, pv_dt=pv_dt,
                     repeats=repeats)
    nc.compile()
    _NC_CACHE[key] = nc
    return nc


def make_in_maps(batch_flat, Wq, bq, Wk):
    bq_r = np.ascontiguousarray(
        np.asarray(bq, dtype=np.float32).reshape(2, P).T
    )
    wq = np.ascontiguousarray(np.asarray(Wq, dtype=np.float32))
    wk = np.ascontiguousarray(np.asarray(Wk, dtype=np.float32))
    return [
        {
            "x": np.ascontiguousarray(np.asarray(batch_flat[n], dtype=np.float32)),
            "wq": wq,
            "wk": wk,
            "bq": bq_r,
        }
        for n in range(N)
    ]


def kernel(batch_flat, Wq, bq, Wk, bk=None, Wv=None, bv=None, **_unused):
    nc = _get_nc()
    in_maps = make_in_maps(batch_flat, Wq, bq, Wk)
    res = bass_utils.run_bass_kernel_spmd(nc, in_maps, core_ids=list(range(N)))
    return np.stack([res.results[n]["out"] for n in range(N)])

